# revision 12
# baseline (speedup 1.0000x reference)
"""Trainium2 Bass kernel for nn_AtomicPositionalEncoding.

kernel(**inputs): FULL x [256,1024,4] f32 -> FULL out [256,1024,128] f32.
Shards batch across 8 NeuronCores (32 examples each), one SPMD Bass program.

Per-core layout: partition p = point-within-tile; global tile j = (b, t),
stored in column order (jq, t, q) where b = 4*jq + q (quads of 4 examples).

Pipeline (per-quad after global stage 1, so phases overlap):
  stage1 -> prodm/prodm2; masks = onehot(cls)
  per quad: hist (PE) -> scale -> scale_quad (PE) -> onehotT (PE transpose)
            -> G = gather via PE matmul -> back to layout A -> PG = prodm*G
            -> mean/istd -> per tile: X=(iota==cls)*PG ; (X-mean)*istd -> DMA
"""

import os
import sys

import numpy as np

for p in ("/opt/trn_rl_repo", "/root/.axon_site/_ro/trn_rl_repo"):
    if os.path.isdir(p) and p not in sys.path:
        sys.path.insert(0, p)

import concourse.bass as bass
import concourse.bacc as bacc
import concourse.mybir as mybir
from concourse.tile import TileContext

F32 = mybir.dt.float32

EX = 32          # examples per core
NPT = 1024       # points per example
T_PER_EX = 8     # tiles of 128 points per example
NCOL = EX * T_PER_EX          # 256 point-tile columns
NQ = 8                        # quads of 4 examples
C = 32
K = 4
D = 128
ETA = 4.0
RC = 6.0
Y00 = 0.5 / np.sqrt(np.pi)
C1 = np.sqrt(3.0 / (4.0 * np.pi))
RS = [0.0, 1.5, 3.0, 4.5]

AF = mybir.ActivationFunctionType
OP = mybir.AluOpType

# ---- tunable engine splits ----
TS_ENG = lambda i: 've' if (i % 16) == 0 else 'act'   # standardize engine
PG_ON_GP = False     # PG=prodm*G multiply on gpsimd (pending probe)
STD_GP_MOD = 0       # if >0: every STD_GP_MOD-th std op on gpsimd


def _consts_f32() -> np.ndarray:
    iota32 = np.tile(np.arange(C, dtype=np.float32), (128, 1))          # [128,32]
    iota128 = np.tile(np.arange(C, dtype=np.float32), (128, K))        # [128,128]
    p_sel = np.zeros((C, 128), dtype=np.float32)                        # [32,128]
    for q in range(4):
        for c in range(C):
            p_sel[c, 32 * q + c] = 1.0
    blockmask = np.zeros((128, 16), dtype=np.float32)                   # [128,16]
    for pp_ in range(128):
        for f in range(16):
            if pp_ // 32 == f // 4:
                blockmask[pp_, f] = 1.0
    ident = np.eye(128, dtype=np.float32)                               # [128,128]
    bconst = np.tile(np.array([np.pi / 2, -1.5, -3.0, -4.5], np.float32), (128, 1))
    return np.concatenate(
        [iota32.ravel(), iota128.ravel(), p_sel.ravel(), blockmask.ravel(), ident.ravel(),
         bconst.ravel()]
    )


CF_SIZES = [128 * 32, 128 * 128, 32 * 128, 128 * 16, 128 * 128, 128 * 4]
CF_TOTAL = sum(CF_SIZES)


def build_nc() -> bass.Bass:
    nc = bacc.Bacc()
    x_d = nc.dram_tensor("x", [EX, NPT, 4], F32, kind="ExternalInput")
    cf_d = nc.dram_tensor("cf", [CF_TOTAL], F32, kind="ExternalInput")
    out_d = nc.dram_tensor("out", [EX, NPT, D], F32, kind="ExternalOutput")

    with TileContext(nc) as tc:
        with (
            tc.tile_pool(name="persist", bufs=1) as pp,
            tc.tile_pool(name="xpool", bufs=6) as xp,
            tc.tile_pool(name="bigbuf", bufs=2) as bb,
            tc.tile_pool(name="outp", bufs=3) as op_,
            tc.tile_pool(name="ph", bufs=2, space="PSUM") as ph,       # hist
            tc.tile_pool(name="poh", bufs=1, space="PSUM") as poh,     # onehotT
            tc.tile_pool(name="pgb", bufs=2, space="PSUM") as pgb,     # G layout-B
            tc.tile_pool(name="pga", bufs=2, space="PSUM") as pga,     # G layout-A
        ):
            ve, act, gp, pe, sy = nc.vector, nc.scalar, nc.gpsimd, nc.tensor, nc.sync

            # ---- constants ----
            offs = np.cumsum([0] + CF_SIZES)
            def cslice(i, shape):
                t = pp.tile(shape, F32, name=f"const{i}", tag=f"const{i}")
                src = cf_d[offs[i]:offs[i + 1]].rearrange("(p f) -> p f", p=shape[0])
                sy.dma_start(t, src)
                return t
            iota32 = cslice(0, [128, 32])
            iota128 = cslice(1, [128, 128])
            p_sel = cslice(2, [32, 128])
            blockmask = cslice(3, [128, 16])
            ident = cslice(4, [128, 128])
            bconst = cslice(5, [128, 4])

            # ---- load x: contiguous staging DMA, then PE-transpose shuffle ----
            # xf[r, 512h+4p+c] = x4[p, 2r+h, c]
            xf = pp.tile([128, NCOL * 4], F32, name="xf", tag="xf")
            sy.dma_start(xf, x_d.rearrange("b n c -> (b n c)")
                             .rearrange("(p f) -> p f", p=128))
            x_sb = pp.tile([128, NCOL * 4], F32, name="x", tag="x")
            x4 = x_sb.rearrange("p (j c) -> p j c", c=4)
            xfv = xf.rearrange("p (h pp c) -> p h pp c", h=2, c=4)
            xcols = x_sb.rearrange("p (r h c) -> p r h c", h=2, c=4)
            for h in range(2):
                for c in range(4):
                    xt_ps = pga.tile([128, 128], F32, name="xt_ps", tag="ga_ps")
                    pe.transpose(xt_ps, xfv[:, h, :, c], ident)
                    if (h * 4 + c) % 2 == 0:
                        ve.tensor_copy(xcols[:, :, h, c], xt_ps)
                    else:
                        act.copy(xcols[:, :, h, c], xt_ps)
            xyz = x4[:, :, 0:3]
            clsf = x4[:, :, 3:4]          # [128,256,1]
            clsf2 = clsf.rearrange("p j one -> p (j one)")  # [128,256]

            # ---- stage 1 ----
            sq = pp.tile([128, NCOL * 3], F32, name="sq", tag="sq").rearrange("p (j c) -> p j c", c=3)
            ve.tensor_tensor(sq, xyz, xyz, OP.mult)
            r2 = pp.tile([128, NCOL], F32, name="r2", tag="r2")
            ve.tensor_reduce(r2, sq, mybir.AxisListType.X, OP.add)
            r = pp.tile([128, NCOL], F32, name="r", tag="r")
            act.activation(r, r2, AF.Sqrt)
            rinv = pp.tile([128, NCOL], F32, name="rinv", tag="rinv")
            ve.reciprocal(rinv, r)
            rmin = pp.tile([128, NCOL], F32, name="rmin", tag="rmin")
            ve.tensor_scalar(out=rmin, in0=r, scalar1=float(RC), scalar2=None, op0=OP.min)
            cosv = pp.tile([128, NCOL], F32, name="cosv", tag="cosv")
            act.activation(cosv, rmin, AF.Sin, bias=bconst[:, 0:1], scale=float(-np.pi / RC))
            env = pp.tile([128, NCOL], F32, name="env", tag="env")
            ve.tensor_scalar(out=env, in0=cosv, scalar1=0.5, scalar2=0.5, op0=OP.mult, op1=OP.add)

            radial = pp.tile([128, NCOL * K], F32, name="radial", tag="radial").rearrange("p (j k) -> p j k", k=K)
            tmp = pp.tile([128, NCOL], F32, name="tmp1", tag="tmp1")
            for k in range(K):
                bias_k = 0.0 if k == 0 else bconst[:, k:k + 1]
                act.activation(tmp, r, AF.Square, bias=bias_k, scale=1.0)
                act.activation(radial[:, :, k:k + 1].rearrange("p j one -> p (j one)"),
                               tmp, AF.Exp, scale=float(-ETA))
            m = pp.tile([128, NCOL * K], F32, name="m", tag="m").rearrange("p (j k) -> p j k", k=K)
            env_b = env.unsqueeze(2).broadcast_to([128, NCOL, K])
            ve.tensor_tensor(m, radial, env_b, OP.mult)

            u3 = pp.tile([128, NCOL * 3], F32, name="u3", tag="u3").rearrange("p (j c) -> p j c", c=3)
            rinv_b = rinv.unsqueeze(2).broadcast_to([128, NCOL, 3])
            ve.tensor_tensor(u3, xyz, rinv_b, OP.mult)
            prodm = pp.tile([128, NCOL * K], F32, name="prodm", tag="prodm").rearrange("p (j k) -> p j k", k=K)
            ve.tensor_scalar(out=prodm[:, :, 0:1], in0=m[:, :, 0:1], scalar1=float(Y00),
                             scalar2=None, op0=OP.mult)
            ve.scalar_tensor_tensor(out=prodm[:, :, 1:3], in0=m[:, :, 1:3], scalar=float(C1),
                                    in1=u3[:, :, 1:3], op0=OP.mult, op1=OP.mult)
            ve.scalar_tensor_tensor(out=prodm[:, :, 3:4], in0=m[:, :, 3:4], scalar=float(C1),
                                    in1=u3[:, :, 0:1], op0=OP.mult, op1=OP.mult)
            prodm2 = pp.tile([128, NCOL * K], F32, name="prodm2", tag="prodm2").rearrange("p (j k) -> p j k", k=K)
            ve.tensor_tensor(prodm2, prodm, prodm, OP.mult)

            # ---- masks: onehot(cls), storage [128, (jq, t, q, c)] ----
            masks_flat = pp.tile([128, NCOL * C], F32, name="masks", tag="masks")
            masks5 = masks_flat.rearrange("p (jq t q c) -> p jq t q c",
                                          jq=NQ, t=T_PER_EX, q=4, c=C)
            iota_b8 = iota32.unsqueeze(1).broadcast_to([128, T_PER_EX, C])
            for b in range(EX):
                jq_, q_ = b // 4, b % 4
                cls_b8 = clsf2[:, 8 * b:8 * b + 8].unsqueeze(2) \
                              .broadcast_to([128, T_PER_EX, C])
                ve.scalar_tensor_tensor(out=masks5[:, jq_, :, q_, :], in0=cls_b8,
                                        scalar=0.0, in1=iota_b8,
                                        op0=OP.add, op1=OP.is_equal)

            # ---- persistent tensors for the quad pipeline ----
            hist_sb = pp.tile([C, 128], F32, name="hist_sb", tag="hist_sb")
            sroot = pp.tile([C, 128], F32, name="sroot", tag="sroot")
            scale_sb = pp.tile([C, 128], F32, name="scale", tag="scale")
            squad = pp.tile([128, 128], F32, name="squad", tag="squad")   # 8x[128,16]
            g_all = pp.tile([128, NQ * 128], F32, name="g_all", tag="g_all")
            pg = pp.tile([128, NCOL * K], F32, name="pg", tag="pg")
            pg3 = pg.rearrange("p (j k) -> p j k", k=K)
            pg5 = pg.rearrange("p (jq t q k) -> p jq t q k", jq=NQ, t=T_PER_EX, q=4)
            g5 = g_all.rearrange("p (jq t q k) -> p jq t q k", jq=NQ, t=T_PER_EX, q=4)
            pg2 = pp.tile([128, NCOL * K], F32, name="pg2", tag="pg2").rearrange("p (j k) -> p j k", k=K)
            mean = pp.tile([128, NCOL], F32, name="mean", tag="mean")
            msq = pp.tile([128, NCOL], F32, name="msq", tag="msq")
            m2 = pp.tile([128, NCOL], F32, name="m2", tag="m2")
            var = pp.tile([128, NCOL], F32, name="var", tag="var")
            std = pp.tile([128, NCOL], F32, name="std", tag="std")
            istd = pp.tile([128, NCOL], F32, name="istd", tag="istd")
            negmistd = pp.tile([128, NCOL], F32, name="negmistd", tag="negmistd")
            iota128_3 = iota128.rearrange("p (k c) -> p k c", c=C)

            # ---- per-quad pipeline ----
            for jq in range(NQ):
                cs = slice(32 * jq, 32 * (jq + 1))        # this quad's 32 cols
                hist_ps = ph.tile([C, 16], F32, name="hist_ps", tag="hist_ps")
                for q in range(4):
                    for t in range(T_PER_EX):
                        pe.matmul(hist_ps[:, 4 * q:4 * q + 4], masks5[:, jq, t, q],
                                  prodm2[:, (4 * jq + q) * T_PER_EX + t],
                                  start=(t == 0), stop=(t == T_PER_EX - 1))
                hs = hist_sb[:, 16 * jq:16 * (jq + 1)]
                ve.tensor_copy(hs, hist_ps)
                sr = sroot[:, 16 * jq:16 * (jq + 1)]
                act.activation(sr, hs, AF.Sqrt)
                sc = scale_sb[:, 16 * jq:16 * (jq + 1)]
                ve.tensor_scalar(out=sc, in0=sr, scalar1=1e-12, scalar2=None, op0=OP.max)
                ve.reciprocal(sc, sc)

                # scale_quad [128,16] = (P_sel.T @ scale_q) * blockmask
                sq_ps = pga.tile([128, 16], F32, name="sq_ps", tag="ga_ps")
                pe.matmul(sq_ps, p_sel, sc, start=True, stop=True)
                sq_sb = squad[:, 16 * jq:16 * (jq + 1)]
                ve.tensor_tensor(sq_sb, sq_ps, blockmask, OP.mult)

                # onehotT [128=(q,c), 1024]
                oh_ps = poh.tile([128, NPT], F32, name="oh_ps", tag="oh_ps")
                for t in range(T_PER_EX):
                    j0 = jq * 32 + t * 4
                    lhs = masks_flat[:, C * j0:C * (j0 + 4)]
                    pe.transpose(oh_ps[:, 128 * t:128 * (t + 1)], lhs, ident)
                oh_sb = bb.tile([128, NPT], F32, name="oh_sb", tag="oh_sb")
                if jq % 2 == 0:
                    ve.tensor_copy(oh_sb, oh_ps)
                else:
                    act.copy(oh_sb, oh_ps)

                # G layout B [16=(q,k), 1024] in two halves, then back to layout A
                gb_sb = bb.tile([16, NPT], F32, name="gb_sb", tag="gb_sb")
                for h in range(2):
                    gb_ps = pgb.tile([16, 512], F32, name="gb_ps", tag="gb_ps")
                    pe.matmul(gb_ps, sq_sb, oh_sb[:, 512 * h:512 * (h + 1)],
                              start=True, stop=True)
                    if h == 0:
                        act.copy(gb_sb[:, 0:512], gb_ps)
                    else:
                        ve.tensor_copy(gb_sb[:, 512:1024], gb_ps)
                ga_ps = pga.tile([128, 128], F32, name="ga_ps", tag="ga_ps")
                for t in range(T_PER_EX):
                    pe.transpose(ga_ps[:, 16 * t:16 * (t + 1)],
                                 gb_sb[:, 128 * t:128 * (t + 1)], ident[:16, :16])
                ve.tensor_copy(g_all[:, 128 * jq:128 * (jq + 1)], ga_ps)

                # PG = prodm * G for this quad (per example)
                for q in range(4):
                    b = 4 * jq + q
                    eng = gp if PG_ON_GP else ve
                    eng.tensor_tensor(pg5[:, jq, :, q, :], prodm[:, 8 * b:8 * b + 8, :],
                                      g5[:, jq, :, q, :], OP.mult)

                # per-point normalization scalars for this quad's 32 cols
                ve.tensor_reduce(mean[:, cs], pg3[:, cs, :], mybir.AxisListType.X, OP.add)
                ve.tensor_scalar(out=mean[:, cs], in0=mean[:, cs],
                                 scalar1=float(1.0 / D), scalar2=None, op0=OP.mult)
                ve.tensor_tensor(pg2[:, cs, :], pg3[:, cs, :], pg3[:, cs, :], OP.mult)
                ve.tensor_reduce(msq[:, cs], pg2[:, cs, :], mybir.AxisListType.X, OP.add)
                act.activation(m2[:, cs], mean[:, cs], AF.Square)
                ve.tensor_scalar(out=msq[:, cs], in0=msq[:, cs],
                                 scalar1=float(1.0 / (D - 1)), scalar2=None, op0=OP.mult)
                ve.scalar_tensor_tensor(out=var[:, cs], in0=m2[:, cs],
                                        scalar=float(-D / (D - 1)), in1=msq[:, cs],
                                        op0=OP.mult, op1=OP.add)
                ve.tensor_scalar(out=var[:, cs], in0=var[:, cs], scalar1=0.0,
                                 scalar2=None, op0=OP.max)
                act.activation(std[:, cs], var[:, cs], AF.Sqrt)
                ve.tensor_scalar(out=std[:, cs], in0=std[:, cs], scalar1=1e-6,
                                 scalar2=None, op0=OP.add)
                ve.reciprocal(istd[:, cs], std[:, cs])
                ve.scalar_tensor_tensor(out=negmistd[:, cs], in0=mean[:, cs], scalar=-1.0,
                                        in1=istd[:, cs], op0=OP.mult, op1=OP.mult)

                # final materialization + output DMA for the 4 examples
                for q in range(4):
                    b = 4 * jq + q
                    out_ex = op_.tile([128, T_PER_EX * D], F32, name="out_ex", tag="out_ex")
                    for t in range(T_PER_EX):
                        jg = jq * 32 + t * 4 + q
                        jc = b * T_PER_EX + t
                        xt = xp.tile([128, D], F32, name="xt", tag="xt")
                        xt3 = xt.rearrange("p (k c) -> p k c", c=C)
                        pg_b = pg3[:, jg, :].unsqueeze(2).broadcast_to([128, K, C])
                        ve.scalar_tensor_tensor(out=xt3, in0=iota128_3,
                                                scalar=clsf2[:, jc:jc + 1], in1=pg_b,
                                                op0=OP.is_equal, op1=OP.mult)
                        dst = out_ex[:, D * t:D * (t + 1)]
                        te = TS_ENG(jc)
                        if STD_GP_MOD and (jc % STD_GP_MOD) == 0:
                            te = 'gp'
                        if te == 'act':
                            act.activation(dst, xt, AF.Identity,
                                           bias=negmistd[:, jg:jg + 1],
                                           scale=istd[:, jg:jg + 1])
                        else:
                            (gp if te == 'gp' else ve).tensor_scalar(
                                out=dst, in0=xt, scalar1=mean[:, jg:jg + 1],
                                scalar2=istd[:, jg:jg + 1],
                                op0=OP.subtract, op1=OP.mult)
                    dst_d = out_d.rearrange("b (t p) j -> b p t j", p=128)[b]
                    sy.dma_start(dst_d, out_ex.rearrange("p (t j) -> p t j", j=D))

    if not nc.is_finalized():
        nc.finalize()
    return nc


_NC = None


def _get_nc():
    global _NC
    if _NC is None:
        _NC = build_nc()
    return _NC


def kernel(x: np.ndarray) -> np.ndarray:
    from concourse.bass_utils import run_bass_kernel_spmd

    x = np.ascontiguousarray(np.asarray(x, dtype=np.float32))
    B = x.shape[0]
    n_cores = 8
    per = B // n_cores
    cf = _consts_f32()
    nc = _get_nc()
    in_maps = [
        {"x": x[i * per:(i + 1) * per], "cf": cf} for i in range(n_cores)
    ]
    res = run_bass_kernel_spmd(nc, in_maps, core_ids=list(range(n_cores)))
    return np.concatenate([r["out"] for r in res.results], axis=0)


if __name__ == "__main__":
    from concourse.bass_interp import CoreSim

    rng = np.random.default_rng(0)
    x = (rng.standard_normal((EX, NPT, 4)) * 2.0).astype(np.float32)
    x[..., 3] = rng.integers(0, C, size=(EX, NPT)).astype(np.float32)
    nc = build_nc()
    sim = CoreSim(nc)
    sim.tensor("x")[:] = x
    sim.tensor("cf")[:] = _consts_f32()
    sim.simulate()
    got = np.array(sim.tensor("out"))

    xyz = x[..., :3]; clsf_ = x[..., 3]
    r = np.sqrt((xyz * xyz).sum(-1)); rinv = 1.0 / r
    radial = np.exp(-ETA * (np.array(RS, np.float32)[None, None] - r[..., None]) ** 2)
    env = 0.5 * np.cos(np.pi * np.minimum(r, RC) / RC) + 0.5
    sh = np.stack([np.full_like(r, Y00), C1 * xyz[..., 1] * rinv,
                   C1 * xyz[..., 2] * rinv, C1 * xyz[..., 0] * rinv], -1)
    prod = sh * radial * env[..., None]
    onehot = (clsf_[..., None] == np.arange(C, dtype=np.float32)).astype(np.float32)
    pos = (prod[..., :, None] * onehot[..., None, :]).reshape(EX, NPT, D)
    norm = np.sqrt((pos * pos).sum(1, keepdims=True))
    pos = pos / np.maximum(norm, 1e-12)
    mean_ = pos.mean(-1, keepdims=True)
    std_ = pos.std(-1, ddof=1, keepdims=True)
    want = (pos - mean_) / (std_ + 1e-6)
    print("sim absmax err:", np.abs(got - want).max(), "ref absmax:", np.abs(want).max())


# revision 14
# speedup vs baseline: 1.1384x; 1.1384x over previous
"""Trainium2 Bass kernel for nn_AtomicPositionalEncoding.

kernel(**inputs): FULL x [256,1024,4] f32 -> FULL out [256,1024,128] f32.
Shards batch across 8 NeuronCores (32 examples each), one SPMD Bass program.

Per-core layout: partition p = point-within-tile; global tile j = (b, t),
stored in column order (jq, t, q) where b = 4*jq + q (quads of 4 examples).

Pipeline (per-quad after global stage 1, so phases overlap):
  stage1 -> prodm/prodm2; masks = onehot(cls)
  per quad: hist (PE) -> scale -> scale_quad (PE) -> onehotT (PE transpose)
            -> G = gather via PE matmul -> back to layout A -> PG = prodm*G
            -> mean/istd -> per tile: X=(iota==cls)*PG ; (X-mean)*istd -> DMA
"""

import os
import sys

import numpy as np

for p in ("/opt/trn_rl_repo", "/root/.axon_site/_ro/trn_rl_repo"):
    if os.path.isdir(p) and p not in sys.path:
        sys.path.insert(0, p)

import concourse.bass as bass
import concourse.bacc as bacc
import concourse.mybir as mybir
from concourse.tile import TileContext

F32 = mybir.dt.float32
BF16 = mybir.dt.bfloat16
F32R = mybir.dt.float32r

EX = 32          # examples per core
NPT = 1024       # points per example
T_PER_EX = 8     # tiles of 128 points per example
NCOL = EX * T_PER_EX          # 256 point-tile columns
NQ = 8                        # quads of 4 examples
C = 32
K = 4
D = 128
ETA = 4.0
RC = 6.0
Y00 = 0.5 / np.sqrt(np.pi)
C1 = np.sqrt(3.0 / (4.0 * np.pi))
RS = [0.0, 1.5, 3.0, 4.5]

AF = mybir.ActivationFunctionType
OP = mybir.AluOpType

# ---- tunable engine splits ----
TS_ENG = lambda i: ('act','act','act','act','act','act','act','act','gp','gp','gp','ve')[i % 12]   # standardize engine
PG_ON_GP = True     # PG=prodm*G multiply on gpsimd (pending probe)
STD_GP_MOD = 0       # if >0: every STD_GP_MOD-th std op on gpsimd


def _consts_f32() -> np.ndarray:
    iota32 = np.tile(np.arange(C, dtype=np.float32), (128, 1))          # [128,32]
    iota128 = np.tile(np.arange(C, dtype=np.float32), (128, K))        # [128,128]
    p_sel = np.zeros((C, 128), dtype=np.float32)                        # [32,128]
    for q in range(4):
        for c in range(C):
            p_sel[c, 32 * q + c] = 1.0
    blockmask = np.zeros((128, 16), dtype=np.float32)                   # [128,16]
    for pp_ in range(128):
        for f in range(16):
            if pp_ // 32 == f // 4:
                blockmask[pp_, f] = 1.0
    ident = np.eye(128, dtype=np.float32)                               # [128,128]
    bconst = np.tile(np.array([np.pi / 2, -1.5, -3.0, -4.5], np.float32), (128, 1))
    return np.concatenate(
        [iota32.ravel(), iota128.ravel(), p_sel.ravel(), blockmask.ravel(), ident.ravel(),
         bconst.ravel()]
    )


CF_SIZES = [128 * 32, 128 * 128, 32 * 128, 128 * 16, 128 * 128, 128 * 4]
CF_TOTAL = sum(CF_SIZES)


def build_nc() -> bass.Bass:
    nc = bacc.Bacc()
    x_d = nc.dram_tensor("x", [EX, NPT, 4], F32, kind="ExternalInput")
    cf_d = nc.dram_tensor("cf", [CF_TOTAL], F32, kind="ExternalInput")
    out_d = nc.dram_tensor("out", [EX, NPT, D], F32, kind="ExternalOutput")

    with TileContext(nc) as tc:
        with (
            tc.tile_pool(name="persist", bufs=1) as pp,
            tc.tile_pool(name="xpool", bufs=6) as xp,
            tc.tile_pool(name="bigbuf", bufs=2) as bb,
            tc.tile_pool(name="outp", bufs=3) as op_,
            tc.tile_pool(name="ph", bufs=2, space="PSUM") as ph,       # hist
            tc.tile_pool(name="poh", bufs=1, space="PSUM") as poh,     # onehotT
            tc.tile_pool(name="pgb", bufs=2, space="PSUM") as pgb,     # G layout-B
            tc.tile_pool(name="pga", bufs=2, space="PSUM") as pga,     # G layout-A
        ):
            ve, act, gp, pe, sy = nc.vector, nc.scalar, nc.gpsimd, nc.tensor, nc.sync

            # ---- constants ----
            offs = np.cumsum([0] + CF_SIZES)
            def cslice(i, shape):
                t = pp.tile(shape, F32, name=f"const{i}", tag=f"const{i}")
                src = cf_d[offs[i]:offs[i + 1]].rearrange("(p f) -> p f", p=shape[0])
                sy.dma_start(t, src)
                return t
            iota32 = cslice(0, [128, 32])
            iota128 = cslice(1, [128, 128])
            p_sel = cslice(2, [32, 128])
            blockmask = cslice(3, [128, 16])
            ident = cslice(4, [128, 128])
            bconst = cslice(5, [128, 4])

            # ---- load x: contiguous staging DMA, then PE-transpose shuffle ----
            # xf[r, 512h+4p+c] = x4[p, 2r+h, c]
            xf = pp.tile([128, NCOL * 4], F32, name="xf", tag="xf")
            sy.dma_start(xf, x_d.rearrange("b n c -> (b n c)")
                             .rearrange("(p f) -> p f", p=128))
            x_sb = pp.tile([128, NCOL * 4], F32, name="x", tag="x")
            x4 = x_sb.rearrange("p (j c) -> p j c", c=4)
            xfv = xf.rearrange("p (h pp c) -> p h pp c", h=2, c=4)
            xcols = x_sb.rearrange("p (r h c) -> p r h c", h=2, c=4)
            for h in range(2):
                for c in range(4):
                    xt_ps = pga.tile([128, 128], F32, name="xt_ps", tag="ga_ps")
                    pe.transpose(xt_ps, xfv[:, h, :, c], ident)
                    if (h * 4 + c) % 2 == 0:
                        ve.tensor_copy(xcols[:, :, h, c], xt_ps)
                    else:
                        act.copy(xcols[:, :, h, c], xt_ps)
            xyz = x4[:, :, 0:3]
            clsf = x4[:, :, 3:4]          # [128,256,1]
            clsf2 = clsf.rearrange("p j one -> p (j one)")  # [128,256]

            # ---- stage 1 ----
            sq = pp.tile([128, NCOL * 3], F32, name="sq", tag="sq").rearrange("p (j c) -> p j c", c=3)
            ve.tensor_tensor(sq, xyz, xyz, OP.mult)
            r2 = pp.tile([128, NCOL], F32, name="r2", tag="r2")
            ve.tensor_reduce(r2, sq, mybir.AxisListType.X, OP.add)
            r = pp.tile([128, NCOL], F32, name="r", tag="r")
            act.activation(r, r2, AF.Sqrt)
            rinv = pp.tile([128, NCOL], F32, name="rinv", tag="rinv")
            ve.reciprocal(rinv, r)
            rmin = pp.tile([128, NCOL], F32, name="rmin", tag="rmin")
            ve.tensor_scalar(out=rmin, in0=r, scalar1=float(RC), scalar2=None, op0=OP.min)
            cosv = pp.tile([128, NCOL], F32, name="cosv", tag="cosv")
            act.activation(cosv, rmin, AF.Sin, bias=bconst[:, 0:1], scale=float(-np.pi / RC))
            env = pp.tile([128, NCOL], F32, name="env", tag="env")
            ve.tensor_scalar(out=env, in0=cosv, scalar1=0.5, scalar2=0.5, op0=OP.mult, op1=OP.add)

            radial = pp.tile([128, NCOL * K], F32, name="radial", tag="radial").rearrange("p (j k) -> p j k", k=K)
            tmp = pp.tile([128, NCOL], F32, name="tmp1", tag="tmp1")
            for k in range(K):
                bias_k = 0.0 if k == 0 else bconst[:, k:k + 1]
                act.activation(tmp, r, AF.Square, bias=bias_k, scale=1.0)
                act.activation(radial[:, :, k:k + 1].rearrange("p j one -> p (j one)"),
                               tmp, AF.Exp, scale=float(-ETA))
            m = pp.tile([128, NCOL * K], F32, name="m", tag="m").rearrange("p (j k) -> p j k", k=K)
            env_b = env.unsqueeze(2).broadcast_to([128, NCOL, K])
            ve.tensor_tensor(m, radial, env_b, OP.mult)

            u3 = pp.tile([128, NCOL * 3], F32, name="u3", tag="u3").rearrange("p (j c) -> p j c", c=3)
            rinv_b = rinv.unsqueeze(2).broadcast_to([128, NCOL, 3])
            ve.tensor_tensor(u3, xyz, rinv_b, OP.mult)
            prodm = pp.tile([128, NCOL * K], F32, name="prodm", tag="prodm").rearrange("p (j k) -> p j k", k=K)
            ve.tensor_scalar(out=prodm[:, :, 0:1], in0=m[:, :, 0:1], scalar1=float(Y00),
                             scalar2=None, op0=OP.mult)
            ve.scalar_tensor_tensor(out=prodm[:, :, 1:3], in0=m[:, :, 1:3], scalar=float(C1),
                                    in1=u3[:, :, 1:3], op0=OP.mult, op1=OP.mult)
            ve.scalar_tensor_tensor(out=prodm[:, :, 3:4], in0=m[:, :, 3:4], scalar=float(C1),
                                    in1=u3[:, :, 0:1], op0=OP.mult, op1=OP.mult)
            prodm2 = pp.tile([128, NCOL * K], BF16, name="prodm2", tag="prodm2").rearrange("p (j k) -> p j k", k=K)
            ve.tensor_tensor(prodm2, prodm, prodm, OP.mult)

            # ---- masks: onehot(cls) in bf16, storage [128, (jq, t, q, c)] ----
            masks_flat = pp.tile([128, NCOL * C], BF16, name="masks", tag="masks")
            masks5 = masks_flat.rearrange("p (jq t q c) -> p jq t q c",
                                          jq=NQ, t=T_PER_EX, q=4, c=C)
            iota_b8 = iota32.unsqueeze(1).broadcast_to([128, T_PER_EX, C])
            ident16 = pp.tile([128, 128], BF16, name="ident16", tag="ident16")
            ve.tensor_copy(ident16, ident)

            # ---- persistent tensors for the quad pipeline ----
            hist_sb = pp.tile([C, 128], F32, name="hist_sb", tag="hist_sb")
            sroot = pp.tile([C, 128], F32, name="sroot", tag="sroot")
            scale_sb = pp.tile([C, 128], F32, name="scale", tag="scale")
            squad = pp.tile([128, 128], F32R, name="squad", tag="squad")   # 8x[128,16]
            g_all = pp.tile([128, NQ * 128], F32, name="g_all", tag="g_all")
            pg = pp.tile([128, NCOL * K], F32, name="pg", tag="pg")
            pg3 = pg.rearrange("p (j k) -> p j k", k=K)
            pg5 = pg.rearrange("p (jq t q k) -> p jq t q k", jq=NQ, t=T_PER_EX, q=4)
            g5 = g_all.rearrange("p (jq t q k) -> p jq t q k", jq=NQ, t=T_PER_EX, q=4)
            pg2 = pp.tile([128, NCOL * K], F32, name="pg2", tag="pg2").rearrange("p (j k) -> p j k", k=K)
            mean = pp.tile([128, NCOL], F32, name="mean", tag="mean")
            msq = pp.tile([128, NCOL], F32, name="msq", tag="msq")
            m2 = pp.tile([128, NCOL], F32, name="m2", tag="m2")
            var = pp.tile([128, NCOL], F32, name="var", tag="var")
            std = pp.tile([128, NCOL], F32, name="std", tag="std")
            istd = pp.tile([128, NCOL], F32, name="istd", tag="istd")
            negmistd = pp.tile([128, NCOL], F32, name="negmistd", tag="negmistd")
            iota128_3 = iota128.rearrange("p (k c) -> p k c", c=C)

            # ---- per-quad pipeline ----
            for jq in range(NQ):
                cs = slice(32 * jq, 32 * (jq + 1))        # this quad's 32 cols
                for q in range(4):
                    b = 4 * jq + q
                    cls_b8 = clsf2[:, 8 * b:8 * b + 8].unsqueeze(2) \
                                  .broadcast_to([128, T_PER_EX, C])
                    ve.scalar_tensor_tensor(out=masks5[:, jq, :, q, :], in0=cls_b8,
                                            scalar=0.0, in1=iota_b8,
                                            op0=OP.add, op1=OP.is_equal)
                hist_ps = ph.tile([C, 16], F32, name="hist_ps", tag="hist_ps")
                for q in range(4):
                    for t in range(T_PER_EX):
                        pe.matmul(hist_ps[:, 4 * q:4 * q + 4], masks5[:, jq, t, q],
                                  prodm2[:, (4 * jq + q) * T_PER_EX + t],
                                  start=(t == 0), stop=(t == T_PER_EX - 1))
                hs = hist_sb[:, 16 * jq:16 * (jq + 1)]
                ve.tensor_copy(hs, hist_ps)
                sr = sroot[:, 16 * jq:16 * (jq + 1)]
                act.activation(sr, hs, AF.Sqrt)
                sc = scale_sb[:, 16 * jq:16 * (jq + 1)]
                ve.tensor_scalar(out=sc, in0=sr, scalar1=1e-12, scalar2=None, op0=OP.max)
                ve.reciprocal(sc, sc)

                # scale_quad [128,16] = (P_sel.T @ scale_q) * blockmask
                sq_ps = pga.tile([128, 16], F32, name="sq_ps", tag="ga_ps")
                pe.matmul(sq_ps, p_sel, sc, start=True, stop=True)
                sq_sb = squad[:, 16 * jq:16 * (jq + 1)]
                ve.tensor_tensor(sq_sb, sq_ps, blockmask, OP.mult)

                # onehotT [128=(q,c), 1024]
                oh_ps = poh.tile([128, NPT], BF16, name="oh_ps", tag="oh_ps")
                for t in range(T_PER_EX):
                    j0 = jq * 32 + t * 4
                    lhs = masks_flat[:, C * j0:C * (j0 + 4)]
                    pe.transpose(oh_ps[:, 128 * t:128 * (t + 1)], lhs, ident16)
                oh_sb = bb.tile([128, NPT], F32R, name="oh_sb", tag="oh_sb")
                if jq % 2 == 0:
                    ve.tensor_copy(oh_sb, oh_ps)
                else:
                    act.copy(oh_sb, oh_ps)

                # G layout B [16=(q,k), 1024] in two halves, then back to layout A
                gb_sb = bb.tile([16, NPT], F32, name="gb_sb", tag="gb_sb")
                for h in range(2):
                    gb_ps = pgb.tile([16, 512], F32, name="gb_ps", tag="gb_ps")
                    pe.matmul(gb_ps, sq_sb,
                              oh_sb[:, 512 * h:512 * (h + 1)],
                              start=True, stop=True)
                    if h == 0:
                        act.copy(gb_sb[:, 0:512], gb_ps)
                    else:
                        ve.tensor_copy(gb_sb[:, 512:1024], gb_ps)
                ga_ps = pga.tile([128, 128], F32, name="ga_ps", tag="ga_ps")
                for t in range(T_PER_EX):
                    pe.transpose(ga_ps[:, 16 * t:16 * (t + 1)],
                                 gb_sb[:, 128 * t:128 * (t + 1)], ident[:16, :16])
                ve.tensor_copy(g_all[:, 128 * jq:128 * (jq + 1)], ga_ps)

                # PG = prodm * G for this quad (per example)
                for q in range(4):
                    b = 4 * jq + q
                    eng = gp if PG_ON_GP else ve
                    eng.tensor_tensor(pg5[:, jq, :, q, :], prodm[:, 8 * b:8 * b + 8, :],
                                      g5[:, jq, :, q, :], OP.mult)

                # per-point normalization scalars for this quad's 32 cols
                ve.tensor_reduce(mean[:, cs], pg3[:, cs, :], mybir.AxisListType.X, OP.add)
                gp.tensor_tensor(pg2[:, cs, :], pg3[:, cs, :], pg3[:, cs, :], OP.mult)
                ve.tensor_reduce(msq[:, cs], pg2[:, cs, :], mybir.AxisListType.X, OP.add)
                act.activation(m2[:, cs], mean[:, cs], AF.Square)
                ve.tensor_scalar(out=msq[:, cs], in0=msq[:, cs],
                                 scalar1=float(1.0 / (D - 1)), scalar2=None, op0=OP.mult)
                ve.scalar_tensor_tensor(out=var[:, cs], in0=m2[:, cs],
                                        scalar=float(-1.0 / (D * (D - 1))), in1=msq[:, cs],
                                        op0=OP.mult, op1=OP.add)
                ve.tensor_scalar(out=var[:, cs], in0=var[:, cs], scalar1=0.0,
                                 scalar2=None, op0=OP.max)
                act.activation(std[:, cs], var[:, cs], AF.Sqrt)
                ve.tensor_scalar(out=std[:, cs], in0=std[:, cs], scalar1=1e-6,
                                 scalar2=None, op0=OP.add)
                ve.reciprocal(istd[:, cs], std[:, cs])
                ve.scalar_tensor_tensor(out=negmistd[:, cs], in0=mean[:, cs],
                                        scalar=float(-1.0 / D),
                                        in1=istd[:, cs], op0=OP.mult, op1=OP.mult)

                # final materialization + output DMA for the 4 examples
                for q in range(4):
                    b = 4 * jq + q
                    out_ex = op_.tile([128, T_PER_EX * D], F32, name="out_ex", tag="out_ex")
                    for t in range(T_PER_EX):
                        jg = jq * 32 + t * 4 + q
                        jc = b * T_PER_EX + t
                        xt = xp.tile([128, D], F32, name="xt", tag="xt")
                        xt3 = xt.rearrange("p (k c) -> p k c", c=C)
                        pg_b = pg3[:, jg, :].unsqueeze(2).broadcast_to([128, K, C])
                        ve.scalar_tensor_tensor(out=xt3, in0=iota128_3,
                                                scalar=clsf2[:, jc:jc + 1], in1=pg_b,
                                                op0=OP.is_equal, op1=OP.mult)
                        dst = out_ex[:, D * t:D * (t + 1)]
                        te = TS_ENG(jc)
                        if STD_GP_MOD and (jc % STD_GP_MOD) == 0:
                            te = 'gp'
                        if te == 'act':
                            act.activation(dst, xt, AF.Identity,
                                           bias=negmistd[:, jg:jg + 1],
                                           scale=istd[:, jg:jg + 1])
                        else:
                            (gp if te == 'gp' else ve).tensor_scalar(
                                out=dst, in0=xt, scalar1=istd[:, jg:jg + 1],
                                scalar2=negmistd[:, jg:jg + 1],
                                op0=OP.mult, op1=OP.add)
                    dst_d = out_d.rearrange("b (t p) j -> b p t j", p=128)[b]
                    sy.dma_start(dst_d, out_ex.rearrange("p (t j) -> p t j", j=D))

    if not nc.is_finalized():
        nc.finalize()
    return nc


_NC = None


def _get_nc():
    global _NC
    if _NC is None:
        _NC = build_nc()
    return _NC


def kernel(x: np.ndarray) -> np.ndarray:
    from concourse.bass_utils import run_bass_kernel_spmd

    x = np.ascontiguousarray(np.asarray(x, dtype=np.float32))
    B = x.shape[0]
    n_cores = 8
    per = B // n_cores
    cf = _consts_f32()
    nc = _get_nc()
    in_maps = [
        {"x": x[i * per:(i + 1) * per], "cf": cf} for i in range(n_cores)
    ]
    res = run_bass_kernel_spmd(nc, in_maps, core_ids=list(range(n_cores)))
    return np.concatenate([r["out"] for r in res.results], axis=0)


if __name__ == "__main__":
    from concourse.bass_interp import CoreSim

    rng = np.random.default_rng(0)
    x = (rng.standard_normal((EX, NPT, 4)) * 2.0).astype(np.float32)
    x[..., 3] = rng.integers(0, C, size=(EX, NPT)).astype(np.float32)
    nc = build_nc()
    sim = CoreSim(nc)
    sim.tensor("x")[:] = x
    sim.tensor("cf")[:] = _consts_f32()
    sim.simulate()
    got = np.array(sim.tensor("out"))

    xyz = x[..., :3]; clsf_ = x[..., 3]
    r = np.sqrt((xyz * xyz).sum(-1)); rinv = 1.0 / r
    radial = np.exp(-ETA * (np.array(RS, np.float32)[None, None] - r[..., None]) ** 2)
    env = 0.5 * np.cos(np.pi * np.minimum(r, RC) / RC) + 0.5
    sh = np.stack([np.full_like(r, Y00), C1 * xyz[..., 1] * rinv,
                   C1 * xyz[..., 2] * rinv, C1 * xyz[..., 0] * rinv], -1)
    prod = sh * radial * env[..., None]
    onehot = (clsf_[..., None] == np.arange(C, dtype=np.float32)).astype(np.float32)
    pos = (prod[..., :, None] * onehot[..., None, :]).reshape(EX, NPT, D)
    norm = np.sqrt((pos * pos).sum(1, keepdims=True))
    pos = pos / np.maximum(norm, 1e-12)
    mean_ = pos.mean(-1, keepdims=True)
    std_ = pos.std(-1, ddof=1, keepdims=True)
    want = (pos - mean_) / (std_ + 1e-6)
    print("sim absmax err:", np.abs(got - want).max(), "ref absmax:", np.abs(want).max())
